# revision 49
# baseline (speedup 1.0000x reference)
"""Multi-head attention block (B=4, S=2048, D=1024, H=16) on 8 TRN2 NeuronCores.

Sharding: core c handles batch b = c//2 and head-group hg = c%2 (8 heads,
a 512-wide slice of the qkv projections). No collectives: each core
computes a [D, S] transposed partial of the output projection for its
head group; the host sums the two head-group partials per batch, adds
the output bias, and transposes back to [S, D].

Per-core dataflow (bf16 compute, f32 PSUM accumulation):
  - host pre-casts all big inputs to bf16 AND pre-transposes q/k/v to
    [D, S] (so the device does no casting and no transposing)
  - Q^T/K^T from projections (dout on partitions); biases folded in as
    ones (x) bias rank-1 matmul updates
  - V in natural [s, dout] layout, augmented with a ones column per head
    (softmax denominators ride along the attn@V matmul as a 65th row)
  - scores^T [k, q] per head via two CONCURRENT K=64 row-tiled matmuls
    (PE tile_position row tiling: even head rows 0:64, odd head rows
    64:128); exp on ACT (PSUM -> SBUF bf16, scale=1/8); O_aug
    accumulated over k tiles in PSUM; normalization via DVE reciprocal +
    GPSIMD partition-broadcast + DVE multiply
  - out^T = Wo^T O^T -> [D, S] f32 -> DMA out
"""

import numpy as np
import ml_dtypes

import concourse.bass as bass
import concourse.bacc as bacc
import concourse.mybir as mybir
from concourse.tile import TileContext
from concourse.bass import ds

F32 = mybir.dt.float32
BF16 = mybir.dt.bfloat16
FP8 = mybir.dt.float8e4
DR = mybir.MatmulPerfMode.DoubleRow
EXP = mybir.ActivationFunctionType.Exp
# Q/K projections run in fp8e4m3 DoubleRow (2x PE throughput). The weights
# are pre-scaled by 16 on the host (uniform(+-1/32) would be half-subnormal
# in e4m3); Q'.K' = 256 * Q.K, folded into the exp scale below.
WSCALE = 16.0

B, S, D, H, HD = 4, 2048, 1024, 16, 64
N_CORES = 8
HPC = H // (N_CORES // B)          # heads per core = 8
DV = HPC * HD                      # 512


def build_attn_core(S=2048, D=1024, HPC=8, HD=64):
    DV = HPC * HD            # head-group width
    NPAIR = HPC // 2         # head pairs; DV = NPAIR * 128
    NDT = D // 128           # din tiles
    NKT = S // 128           # key tiles
    QC = 512                 # q chunk
    NQC = S // QC
    SC = 512                 # s chunk for projections
    NSC = S // SC
    SCALE = HD ** -0.5

    nc = bacc.Bacc("TRN2", target_bir_lowering=False)
    q_ext = nc.dram_tensor("queryT", [D, S], FP8, kind="ExternalInput")
    k_ext = nc.dram_tensor("keyT", [D, S], FP8, kind="ExternalInput")
    v_ext = nc.dram_tensor("valueT", [D, S], BF16, kind="ExternalInput")
    wq_ext = nc.dram_tensor("Wq", [D, DV], FP8, kind="ExternalInput")
    wk_ext = nc.dram_tensor("Wk", [D, DV], FP8, kind="ExternalInput")
    wv_ext = nc.dram_tensor("Wv", [D, DV], BF16, kind="ExternalInput")
    wo_ext = nc.dram_tensor("Wo", [DV, D], BF16, kind="ExternalInput")
    bq_ext = nc.dram_tensor("bq", [DV], BF16, kind="ExternalInput")
    bk_ext = nc.dram_tensor("bk", [DV], BF16, kind="ExternalInput")
    bv_ext = nc.dram_tensor("bv", [DV], BF16, kind="ExternalInput")
    # bf16 output halves the 8MB per-core result DMA; the host sums the
    # two head-group partials in f32 and adds the f32 bias, so the only
    # extra error is one bf16 rounding of each partial (~0.2% RMS).
    out_ext = nc.dram_tensor("out", [D, S], BF16, kind="ExternalOutput")

    with TileContext(nc) as tc:
        with (
            tc.tile_pool(name="const", bufs=1) as cpool,
            tc.tile_pool(name="big", bufs=1) as big,
            tc.tile_pool(name="pt", bufs=8) as ptpool,
            tc.tile_pool(name="vl", bufs=6) as vlpool,
            tc.tile_pool(name="rec", bufs=2) as recpool,
            tc.tile_pool(name="oun", bufs=4) as ounpool,
            tc.tile_pool(name="stage", bufs=4) as stage,
            tc.tile_pool(name="mmps", bufs=2, space="PSUM") as mmps,
            tc.tile_pool(name="scps", bufs=2, space="PSUM") as scps,
            tc.tile_pool(name="ops", bufs=2, space="PSUM") as opool,
        ):
            # -------- biases / ones first (tiny DMAs; the last matmul of
            # every projection group needs them, so they must not queue
            # behind the big transfers). Zero-padded to 128 partitions so
            # every matmul runs in the same 128x128 tile mode.
            bq_pad = cpool.tile([128, DV], BF16, tag="bqp")
            bk_pad = cpool.tile([128, DV], BF16, tag="bkp")
            bv_pad = cpool.tile([128, DV], BF16, tag="bvp")
            ones_pad = cpool.tile([128, SC], BF16, tag="onesp")
            nc.vector.memset(bq_pad[:], 0.0)
            nc.vector.memset(bk_pad[:], 0.0)
            nc.vector.memset(bv_pad[:], 0.0)
            nc.vector.memset(ones_pad[:], 0.0)
            nc.vector.memset(ones_pad[0:1, :], 1.0)
            nc.sync.dma_start(bq_pad[0:1, :], bq_ext.rearrange("(a n) -> a n", a=1))
            nc.sync.dma_start(bk_pad[0:1, :], bk_ext.rearrange("(a n) -> a n", a=1))
            nc.sync.dma_start(bv_pad[0:1, :], bv_ext.rearrange("(a n) -> a n", a=1))

            # -------- big inputs: emission order = DMA priority ----------
            # inputs arrive pre-transposed ([D, S]) from the host, so X^T
            # loads are plain large DMAs; V's stationary tiles stream
            # on-demand from DRAM inside v_proj_st.
            xqT = big.tile([128, NDT, S], FP8, tag="xqT")
            xkT = big.tile([128, NDT, S], FP8, tag="xkT")
            wq_sb = big.tile([128, NDT, DV], FP8, tag="wq")
            wk_sb = big.tile([128, NDT, DV], FP8, tag="wk")
            wv_sb = big.tile([128, NDT, DV], BF16, tag="wv")
            wo_sb = big.tile([128, NPAIR, D], BF16, tag="wo")
            # xqT/xkT split into halves along S so the first projection
            # chunks can start before the full 4MB tensor lands; wv hoisted
            # before xkT's second half so the early v_proj fillers in
            # attention(0) aren't DMA-starved.
            # Arrival order tracks first use: attention(0, qc0) needs only
            # q(0,sc0) and progressively more of kT and V. The second input
            # halves (xk-h2, xq-h2) and wo are issued later, interleaved
            # with the early V-slice prefetches, so nothing the first chunk
            # consumes queues behind bytes it does not.
            HS = S // 2
            nc.sync.dma_start(wq_sb[:], wq_ext.rearrange("(t p) n -> p t n", p=128))
            nc.sync.dma_start(
                xqT[:, :, 0:HS], q_ext[:, 0:HS].rearrange("(t p) s -> p t s", p=128)
            )
            nc.sync.dma_start(wk_sb[:], wk_ext.rearrange("(t p) n -> p t n", p=128))
            nc.sync.dma_start(
                xkT[:, :, 0:HS], k_ext[:, 0:HS].rearrange("(t p) s -> p t s", p=128)
            )
            nc.sync.dma_start(wv_sb[:], wv_ext.rearrange("(t p) n -> p t n", p=128))

            def dma_xk_h2():
                nc.sync.dma_start(
                    xkT[:, :, HS:S],
                    k_ext[:, HS:S].rearrange("(t p) s -> p t s", p=128),
                )

            def dma_xq_h2():
                nc.sync.dma_start(
                    xqT[:, :, HS:S],
                    q_ext[:, HS:S].rearrange("(t p) s -> p t s", p=128),
                )

            def dma_wo():
                nc.sync.dma_start(
                    wo_sb[:], wo_ext.rearrange("(t p) n -> p t n", p=128)
                )

            # -------- persistent SBUF tensors ----------------------------
            # qT/kT hold the head pair stacked on partitions (even head on
            # 0:64, odd head on 64:128); the scores matmuls use K=64
            # row-tiles at base partitions 0 and 64, which run concurrently
            # on disjoint PE array cells (tile_position row tiling).
            qT = big.tile([128, NPAIR, S], BF16, tag="qT")
            kT = big.tile([128, NPAIR, S], BF16, tag="kT")
            v_aug = big.tile([128, NKT, HPC * 65], BF16, tag="vaug")
            # own memory (not aliased to xqT): later pairs' q-projections are
            # interleaved into attention as fillers, so xqT stays live while
            # oT is being written
            oT = big.tile([128, NPAIR, S], BF16, tag="oT")

            # -------- work-chunk emitters --------------------------------
            def q_proj_sc(t, sc, pool=None):
                pool, tag = pool or (mmps, "mm")
                ps = pool.tile([128, SC], F32, tag=tag)
                for dk2 in range(NDT // 2):
                    nc.tensor.matmul(
                        ps[:],
                        wq_sb[:, 2 * dk2 : 2 * dk2 + 2, ds(t * 128, 128)],
                        xqT[:, 2 * dk2 : 2 * dk2 + 2, ds(sc * SC, SC)],
                        start=(dk2 == 0),
                        stop=False,
                        perf_mode=DR,
                    )
                nc.tensor.matmul(
                    ps[:], bq_pad[:, ds(t * 128, 128)], ones_pad[:],
                    start=False, stop=True,
                )
                nc.vector.tensor_copy(qT[:, t, ds(sc * SC, SC)], ps[:])

            def k_proj_sc(t, sc, pool=None):
                pool, tag = pool or (mmps, "mm")
                ps = pool.tile([128, SC], F32, tag=tag)
                for dk2 in range(NDT // 2):
                    nc.tensor.matmul(
                        ps[:],
                        wk_sb[:, 2 * dk2 : 2 * dk2 + 2, ds(t * 128, 128)],
                        xkT[:, 2 * dk2 : 2 * dk2 + 2, ds(sc * SC, SC)],
                        start=(dk2 == 0),
                        stop=False,
                        perf_mode=DR,
                    )
                nc.tensor.matmul(
                    ps[:], bk_pad[:, ds(t * 128, 128)], ones_pad[:],
                    start=False, stop=True,
                )
                nc.vector.tensor_copy(kT[:, t, ds(sc * SC, SC)], ps[:])

            vl_tiles = {}

            def v_dma_st(st):
                vl = vlpool.tile([128, NDT, 128], BF16, tag="vl")
                nc.sync.dma_start(
                    vl[:],
                    v_ext[:, ds(st * 128, 128)].rearrange("(t p) s -> p t s", p=128),
                )
                vl_tiles[st] = vl

            def v_proj_st(st):
                vl = vl_tiles.pop(st)
                ps = mmps.tile([128, DV], F32, tag="mm")
                for dk in range(NDT):
                    nc.tensor.matmul(
                        ps[:],
                        vl[:, dk, :],
                        wv_sb[:, dk, :],
                        start=(dk == 0),
                        stop=False,
                    )
                nc.tensor.matmul(
                    ps[:], ones_pad[:, 0:128], bv_pad[:], start=False, stop=True
                )
                dst = v_aug[:, st, :].rearrange("p (h c) -> p h c", c=65)
                nc.vector.tensor_copy(
                    dst[:, :, 0:64], ps[:].rearrange("p (h c) -> p h c", c=64)
                )
                nc.vector.memset(dst[:, :, 64:65], 1.0)

            def outproj_dt(sc, dt2, c0=0, c1=None):
                c1 = SC if c1 is None else c1
                w = c1 - c0
                po = mmps.tile([128, SC], F32, tag="mm")
                for ht in range(NPAIR):
                    nc.tensor.matmul(
                        po[:, 0:w],
                        wo_sb[:, ht, ds(dt2 * 128, 128)],
                        oT[:, ht, ds(sc * SC + c0, w)],
                        start=(ht == 0),
                        stop=(ht == NPAIR - 1),
                    )
                ost = stage.tile([128, SC], BF16, tag="ostage")
                nc.vector.tensor_copy(ost[:, 0:w], po[:, 0:w])
                nc.sync.dma_start(
                    out_ext[ds(dt2 * 128, 128), ds(sc * SC + c0, w)], ost[:, 0:w]
                )

            # -------- HAM warm-up ----------------------------------------
            # ~7us of dependency-free matmuls on the ones tile: keeps the PE
            # busy during the initial DMA-only window so the HAM activity
            # monitor un-throttles the clock before the real projections.
            for wi in range(18):
                wps = mmps.tile([128, SC], F32, tag="mm")
                for _ in range(2):
                    nc.tensor.matmul(
                        wps[:], ones_pad[:, 0:128], ones_pad[:],
                        start=True, stop=True,
                    )

            # -------- projections needed before attention(0) -------------
            # Rotate PSUM groups through the (still idle) attention pools so
            # the PE pipeline stays dense -- otherwise the ~1.4us slot-wait
            # gaps keep resetting the HAM activity window and the whole
            # projection phase runs at half clock.
            rot = [(mmps, "mm"), (scps, "sc"), (opool, "o")]
            ri = 0
            # minimal pre-work: attention(0, qc0) only needs q chunk 0 and
            # (progressively) kT; the rest stream in as fillers. The first
            # V-slice prefetches are issued here so they land before the
            # second input halves in the DMA queue.
            q_proj_sc(0, 0, pool=rot[0])
            k_proj_sc(0, 0, pool=rot[1])
            for st in range(3):
                v_dma_st(st)
            dma_xk_h2()

            # -------- attention ------------------------------------------
            # ACT (exp) is the long pole; every other PE-work chunk is
            # interleaved into the kt loop as "filler" so the in-order PE
            # queue never parks a long burst in front of the next scores.
            # WSCALE**2 undoes the host-side fp8 weight pre-scaling of Wq/Wk.
            SCALEF = SCALE / (WSCALE * WSCALE)

            deferred = []

            def normalize_one(o_sb, t, qc, hh, c0=0, c1=None):
                # Move the denominator row to partition 0 with a tiny
                # SBUF->SBUF DMA (partition-free, idle engines), broadcast,
                # then take the reciprocal on all 64 lanes in parallel --
                # the HW partition-broadcast only reads partition 0 (an AP
                # base of 64 yields NaN), and a [1, N] reciprocal runs
                # serially on one DVE lane.
                c1 = QC if c1 is None else c1
                w = c1 - c0
                dn = recpool.tile([64, QC], F32, tag="rec")
                if o_sb.space == bass.MemorySpace.PSUM:
                    # DMA cannot read PSUM; a [1, N] DVE copy does the row
                    # move (single lane, ~0.6us) without the sync-queue hop
                    nc.vector.tensor_copy(dn[0:1, 0:w], o_sb[64:65, c0:c1])
                else:
                    nc.sync.dma_start(dn[0:1, 0:w], o_sb[64:65, c0:c1])
                bc = recpool.tile([64, QC], F32, tag="rec")
                nc.gpsimd.partition_broadcast(bc[:, 0:w], dn[0:1, 0:w])
                # ~51 ULP, ~5x faster than reciprocal() (which costs ~3.3us
                # per 512 columns regardless of partition count)
                nc.vector.reciprocal_approx_fast(out=bc[:, 0:w], in_=bc[:, 0:w])
                nc.vector.tensor_mul(
                    oT[ds(hh * 64, 64), t, ds(qc * QC + c0, w)],
                    o_sb[0:64, c0:c1],
                    bc[:, 0:w],
                )

            def attention_pair(t):
                for qc in range(NQC):
                    # filler thunks interleaved after each scores/exp step;
                    # lag = how many k-tiles attn@V trails the exp stream
                    # (deep for the very first chunk so attn@V can wait for
                    # the V projection without stalling the exp feed)
                    fillers = {}
                    lag = 4
                    if t == 0 and qc == 0:
                        # V-slice DMA prefetch ~3 k-tiles ahead of its
                        # projection; projection one kt ahead of attn@V's lag
                        for st in range(3, NKT):
                            fillers.setdefault(min(st - 2, NKT - 1), []).append(
                                lambda st=st: v_dma_st(st)
                            )
                        for st in range(NKT):
                            fillers.setdefault(min(1 + st, NKT - 1), []).append(
                                lambda st=st: v_proj_st(st)
                            )
                        # rest of pair-0's K projection, each chunk one block
                        # ahead of the first scores k-tile that reads it
                        for j, slot in ((1, 2), (2, 5), (3, 9)):
                            fillers.setdefault(slot, []).append(
                                lambda j=j: k_proj_sc(0, j)
                            )
                        # xq second half: needed by q(0,2) emitted at kt 11
                        fillers.setdefault(8, []).append(dma_xq_h2)
                    if t == 0 and qc == 1:
                        fillers.setdefault(0, []).append(dma_wo)
                    if t == 0 and qc < NQC - 1:
                        # pair-0's next q chunk, late in the current chunk
                        fillers.setdefault(11, []).append(
                            lambda sc=qc + 1: q_proj_sc(0, sc)
                        )
                    if t + 1 < NPAIR and qc in (1, 2):
                        # next pair's Q projection, two chunks per qc chunk
                        for i in range(2):
                            fillers.setdefault(3 + 8 * i, []).append(
                                lambda tt=t + 1, sc=2 * (qc - 1) + i: q_proj_sc(tt, sc)
                            )
                    if t + 1 < NPAIR and qc == NQC - 1:
                        for i in range(NSC):
                            fillers.setdefault(2 + 4 * i, []).append(
                                lambda tt=t + 1, sc=i: k_proj_sc(tt, sc)
                            )
                    if t == NPAIR - 1 and qc > 0:
                        # kt >= 8 (block 4+) only: after BOTH deferred
                        # normalizations of the previous chunk (popped in
                        # block 1) have finished their DVE chains and written
                        # the oT slices these consume
                        for i in range(NDT):
                            fillers.setdefault(8 + i, []).append(
                                lambda sc=qc - 1, dt2=i: outproj_dt(sc, dt2)
                            )

                    oA = opool.tile([65, QC], F32, tag="o")
                    oB = opool.tile([65, QC], F32, tag="o")
                    pts = {}

                    def scores_exp(kt):
                        # two concurrent K=64 row-tiles: even head on array
                        # rows 0:64, odd head on rows 64:128 (tile_position
                        # inferred from the operands' base partitions)
                        sct = scps.tile([128, 2 * QC], F32, tag="sc")
                        nc.tensor.matmul(
                            sct[:, 0:QC],
                            kT[0:64, t, ds(kt * 128, 128)],
                            qT[0:64, t, ds(qc * QC, QC)],
                            start=True, stop=True,
                        )
                        nc.tensor.matmul(
                            sct[:, QC : 2 * QC],
                            kT[64:128, t, ds(kt * 128, 128)],
                            qT[64:128, t, ds(qc * QC, QC)],
                            start=True, stop=True,
                        )
                        pt = ptpool.tile([128, 2 * QC], BF16, tag="pt")
                        nc.scalar.activation(pt[:], sct[:], EXP, bias=0.0, scale=SCALEF)
                        pts[kt] = pt

                    def attn_v(kt):
                        pt = pts.pop(kt)
                        nc.tensor.matmul(
                            oA[:],
                            v_aug[:, kt, ds((2 * t) * 65, 65)],
                            pt[:, 0:QC],
                            start=(kt == 0),
                            stop=(kt == NKT - 1),
                        )
                        nc.tensor.matmul(
                            oB[:],
                            v_aug[:, kt, ds((2 * t + 1) * 65, 65)],
                            pt[:, QC : 2 * QC],
                            start=(kt == 0),
                            stop=(kt == NKT - 1),
                        )

                    # 2-kt blocks, mode-batched: per block the PE sees one
                    # 128-mode burst (attn@V pair + fillers) then one 64-mode
                    # burst (2x row-tiled scores pairs), so tile-mode drains
                    # happen twice per block instead of twice per kt. Scores
                    # go LAST in the block: they are the instructions that can
                    # stall on the ACT exp (scps WAR), and the in-order PE
                    # queue must have the fillers in front of them.
                    # In the very first chunk the fillers are DMA-gated
                    # (V slices still landing), so scores go FIRST there to
                    # keep the exp stream fed; everywhere else scores go last
                    # because they are the ops that stall on the ACT (scps
                    # WAR) and the in-order PE queue must keep fillers in
                    # front of them.
                    first = t == 0 and qc == 0
                    for b in range(NKT // 2):
                        kt0, kt1 = 2 * b, 2 * b + 1
                        if kt0 - lag >= 0:
                            attn_v(kt0 - lag)
                            attn_v(kt1 - lag)
                        if first:
                            scores_exp(kt0)
                            scores_exp(kt1)
                        for kt in (kt0, kt1):
                            for f in fillers.get(kt, ()):
                                f()
                        if b == 1:
                            while deferred:
                                normalize_one(*deferred.pop(0))
                        if not first:
                            scores_exp(kt0)
                            scores_exp(kt1)
                    for kt in range(max(0, NKT - lag), NKT):
                        attn_v(kt)

                    # copy both accumulators PSUM->SBUF now (frees the
                    # banks for the next chunk within ~1us); the recip/
                    # broadcast/mul chains are DEFERRED into the next
                    # chunk's loop so they soak up DVE idle time there
                    # instead of stalling this boundary
                    for o_ps, hh in ((oA, 0), (oB, 1)):
                        if t == NPAIR - 1 and qc == NQC - 1:
                            # final chunk: normalize straight from PSUM (no
                            # further matmuls will claim these banks), which
                            # drops two DVE copies from the closing chain
                            deferred.append((o_ps, t, qc, hh))
                        else:
                            o_sb = ounpool.tile([65, QC], F32, tag="oun")
                            nc.vector.tensor_copy(o_sb[:], o_ps[:])
                            deferred.append((o_sb, t, qc, hh))

            for t in range(NPAIR):
                attention_pair(t)
            # Tail: the final chunk's two head-norms are still deferred.
            # Process them in two column waves (384 then 128) with the
            # output projection interleaved, so the last serial
            # norm->outproj->DMA chain only covers 128 columns.
            SPLIT = 384
            final = list(deferred)
            deferred.clear()
            for args in final:
                normalize_one(*args, 0, SPLIT)
            # norm-b emitted before the first wave: its DVE/gpsimd chain
            # overlaps the 384-wave matmuls instead of queueing behind
            # their staging copies, so the 128-wave starts immediately
            for args in final:
                normalize_one(*args, SPLIT, QC)
            for dt2 in range(NDT):
                outproj_dt(NQC - 1, dt2, 0, SPLIT)
            for dt2 in range(NDT):
                outproj_dt(NQC - 1, dt2, SPLIT, SC)

    nc.finalize()
    return nc


_NC_CACHE = {}


def _get_nc():
    if "nc" not in _NC_CACHE:
        _NC_CACHE["nc"] = build_attn_core(S=S, D=D, HPC=HPC, HD=HD)
    return _NC_CACHE["nc"]


def _make_in_maps(query, key, value, Wq, bq, Wk, bk, Wv, bv, Wo):
    bf = ml_dtypes.bfloat16
    f8 = ml_dtypes.float8_e4m3fn
    ws = 16.0  # must match kernel WSCALE
    in_maps = []
    for c in range(N_CORES):
        b, hg = c // 2, c % 2
        sl = slice(hg * DV, (hg + 1) * DV)
        in_maps.append(dict(
            queryT=np.ascontiguousarray(query[b].T).astype(f8),
            keyT=np.ascontiguousarray(key[b].T).astype(f8),
            valueT=np.ascontiguousarray(value[b].T).astype(bf),
            Wq=(np.ascontiguousarray(Wq[:, sl]) * ws).astype(f8),
            Wk=(np.ascontiguousarray(Wk[:, sl]) * ws).astype(f8),
            Wv=np.ascontiguousarray(Wv[:, sl]).astype(bf),
            Wo=np.ascontiguousarray(Wo[sl, :]).astype(bf),
            bq=(np.ascontiguousarray(bq[sl]) * ws).astype(bf),
            bk=(np.ascontiguousarray(bk[sl]) * ws).astype(bf),
            bv=np.ascontiguousarray(bv[sl]).astype(bf),
        ))
    return in_maps


def _assemble(results, bo):
    out = np.empty((B, S, D), dtype=np.float32)
    for b in range(B):
        part = (results[2 * b]["out"].astype(np.float32)
                + results[2 * b + 1]["out"].astype(np.float32))   # [D, S]
        out[b] = part.T + bo
    return out


def run(inputs, trace=False):
    """Run on 8 cores; returns (output, BassKernelResults)."""
    from concourse.bass_utils import run_bass_kernel_spmd

    inputs = {k: np.asarray(v) for k, v in inputs.items()}
    nc = _get_nc()
    in_maps = _make_in_maps(
        inputs["query"], inputs["key"], inputs["value"],
        inputs["Wq"], inputs["bq"], inputs["Wk"], inputs["bk"],
        inputs["Wv"], inputs["bv"], inputs["Wo"],
    )
    res = run_bass_kernel_spmd(
        nc, in_maps, core_ids=list(range(N_CORES)), trace=trace
    )
    out = _assemble(res.results, np.asarray(inputs["bo"], dtype=np.float32))
    return out, res


def kernel(**inputs) -> np.ndarray:
    out, _ = run(inputs, trace=False)
    return out



# revision 57
# speedup vs baseline: 1.0256x; 1.0256x over previous
"""Multi-head attention block (B=4, S=2048, D=1024, H=16) on 8 TRN2 NeuronCores.

Sharding: core c handles batch b = c//2 and head-group hg = c%2 (8 heads,
a 512-wide slice of the qkv projections). No collectives: each core
computes a [D, S] transposed partial of the output projection for its
head group; the host sums the two head-group partials per batch, adds
the output bias, and transposes back to [S, D].

Per-core dataflow (bf16 compute, f32 PSUM accumulation):
  - host pre-casts all big inputs to bf16 AND pre-transposes q/k/v to
    [D, S] (so the device does no casting and no transposing)
  - Q^T/K^T from projections (dout on partitions); biases folded in as
    ones (x) bias rank-1 matmul updates
  - V in natural [s, dout] layout, augmented with a ones column per head
    (softmax denominators ride along the attn@V matmul as a 65th row)
  - scores^T [k, q] per head via two CONCURRENT K=64 row-tiled matmuls
    (PE tile_position row tiling: even head rows 0:64, odd head rows
    64:128); exp on ACT (PSUM -> SBUF bf16, scale=1/8); O_aug
    accumulated over k tiles in PSUM; normalization via DVE reciprocal +
    GPSIMD partition-broadcast + DVE multiply
  - out^T = Wo^T O^T -> [D, S] f32 -> DMA out
"""

import numpy as np
import ml_dtypes

import concourse.bass as bass
import concourse.bacc as bacc
import concourse.mybir as mybir
from concourse.tile import TileContext
from concourse.bass import ds

F32 = mybir.dt.float32
BF16 = mybir.dt.bfloat16
FP8 = mybir.dt.float8e4
DR = mybir.MatmulPerfMode.DoubleRow
EXP = mybir.ActivationFunctionType.Exp
# Q/K projections run in fp8e4m3 DoubleRow (2x PE throughput). The weights
# are pre-scaled by 16 on the host (uniform(+-1/32) would be half-subnormal
# in e4m3); Q'.K' = 256 * Q.K, folded into the exp scale below.
WSCALE = 16.0

B, S, D, H, HD = 4, 2048, 1024, 16, 64
N_CORES = 8
HPC = H // (N_CORES // B)          # heads per core = 8
DV = HPC * HD                      # 512


def build_attn_core(S=2048, D=1024, HPC=8, HD=64):
    DV = HPC * HD            # head-group width
    NPAIR = HPC // 2         # head pairs; DV = NPAIR * 128
    NDT = D // 128           # din tiles
    NKT = S // 128           # key tiles
    QC = 512                 # q chunk
    NQC = S // QC
    SC = 512                 # s chunk for projections
    NSC = S // SC
    SCALE = HD ** -0.5

    nc = bacc.Bacc("TRN2", target_bir_lowering=False)
    q_ext = nc.dram_tensor("queryT", [D, S], FP8, kind="ExternalInput")
    k_ext = nc.dram_tensor("keyT", [D, S], FP8, kind="ExternalInput")
    v_ext = nc.dram_tensor("valueT", [D, S], BF16, kind="ExternalInput")
    wq_ext = nc.dram_tensor("Wq", [D, DV], FP8, kind="ExternalInput")
    wk_ext = nc.dram_tensor("Wk", [D, DV], FP8, kind="ExternalInput")
    wv_ext = nc.dram_tensor("Wv", [D, DV], BF16, kind="ExternalInput")
    wo_ext = nc.dram_tensor("Wo", [DV, D], BF16, kind="ExternalInput")
    bq_ext = nc.dram_tensor("bq", [DV], BF16, kind="ExternalInput")
    bk_ext = nc.dram_tensor("bk", [DV], BF16, kind="ExternalInput")
    # bf16 output halves the 8MB per-core result DMA; the host sums the
    # two head-group partials in f32 and adds the f32 bias, so the only
    # extra error is one bf16 rounding of each partial (~0.2% RMS).
    out_ext = nc.dram_tensor("out", [D, S], BF16, kind="ExternalOutput")

    with TileContext(nc) as tc:
        with (
            tc.tile_pool(name="const", bufs=1) as cpool,
            tc.tile_pool(name="big", bufs=1) as big,
            tc.tile_pool(name="pt", bufs=8) as ptpool,
            tc.tile_pool(name="vl", bufs=6) as vlpool,
            tc.tile_pool(name="rec", bufs=2) as recpool,
            tc.tile_pool(name="oun", bufs=4) as ounpool,
            tc.tile_pool(name="stage", bufs=4) as stage,
            tc.tile_pool(name="mmps", bufs=2, space="PSUM") as mmps,
            tc.tile_pool(name="scps", bufs=2, space="PSUM") as scps,
            tc.tile_pool(name="ops", bufs=2, space="PSUM") as opool,
        ):
            # -------- biases / ones first (tiny DMAs; the last matmul of
            # every projection group needs them, so they must not queue
            # behind the big transfers). Zero-padded to 128 partitions so
            # every matmul runs in the same 128x128 tile mode.
            # biases in column layout [dout-partition, t-tile]: they ride the
            # projection's PSUM->SBUF copy as a DVE tensor_scalar add (one
            # per-partition vector per t-tile), no bias matmuls. bv is folded
            # into the host-side output bias (bo' = bv @ Wo + bo) entirely.
            bq_bf = cpool.tile([128, NPAIR], BF16, tag="bqb")
            bk_bf = cpool.tile([128, NPAIR], BF16, tag="bkb")
            bq_col = cpool.tile([128, NPAIR], F32, tag="bqc")
            bk_col = cpool.tile([128, NPAIR], F32, tag="bkc")
            ones_pad = cpool.tile([128, SC], BF16, tag="onesp")
            nc.vector.memset(ones_pad[:], 0.0)
            nc.vector.memset(ones_pad[0:1, :], 1.0)
            nc.sync.dma_start(bq_bf[:], bq_ext.rearrange("(t p) -> p t", p=128))
            nc.sync.dma_start(bk_bf[:], bk_ext.rearrange("(t p) -> p t", p=128))
            # tensor_scalar's AP operand must be f32: cast once
            nc.vector.tensor_copy(bq_col[:], bq_bf[:])
            nc.vector.tensor_copy(bk_col[:], bk_bf[:])

            # -------- big inputs: emission order = DMA priority ----------
            # inputs arrive pre-transposed ([D, S]) from the host, so X^T
            # loads are plain large DMAs; V's stationary tiles stream
            # on-demand from DRAM inside v_proj_st.
            xqT = big.tile([128, NDT, S], FP8, tag="xqT")
            xkT = big.tile([128, NDT, S], FP8, tag="xkT")
            wq_sb = big.tile([128, NDT, DV], FP8, tag="wq")
            wk_sb = big.tile([128, NDT, DV], FP8, tag="wk")
            wv_sb = big.tile([128, NDT, DV], BF16, tag="wv")
            wo_sb = big.tile([128, NPAIR, D], BF16, tag="wo")
            # xqT/xkT split into halves along S so the first projection
            # chunks can start before the full 4MB tensor lands; wv hoisted
            # before xkT's second half so the early v_proj fillers in
            # attention(0) aren't DMA-starved.
            # Arrival order tracks first use: attention(0, qc0) needs only
            # q(0,sc0) and progressively more of kT and V. The second input
            # halves (xk-h2, xq-h2) and wo are issued later, interleaved
            # with the early V-slice prefetches, so nothing the first chunk
            # consumes queues behind bytes it does not.
            HS = S // 2
            nc.sync.dma_start(wq_sb[:], wq_ext.rearrange("(t p) n -> p t n", p=128))
            nc.sync.dma_start(
                xqT[:, :, 0:HS], q_ext[:, 0:HS].rearrange("(t p) s -> p t s", p=128)
            )
            nc.sync.dma_start(wk_sb[:], wk_ext.rearrange("(t p) n -> p t n", p=128))
            nc.sync.dma_start(
                xkT[:, :, 0:HS], k_ext[:, 0:HS].rearrange("(t p) s -> p t s", p=128)
            )
            nc.sync.dma_start(wv_sb[:], wv_ext.rearrange("(t p) n -> p t n", p=128))

            def dma_xk_h2():
                nc.sync.dma_start(
                    xkT[:, :, HS:S],
                    k_ext[:, HS:S].rearrange("(t p) s -> p t s", p=128),
                )

            def dma_xq_h2():
                nc.sync.dma_start(
                    xqT[:, :, HS:S],
                    q_ext[:, HS:S].rearrange("(t p) s -> p t s", p=128),
                )

            def dma_wo():
                nc.sync.dma_start(
                    wo_sb[:], wo_ext.rearrange("(t p) n -> p t n", p=128)
                )

            # -------- persistent SBUF tensors ----------------------------
            # qT/kT hold the head pair stacked on partitions (even head on
            # 0:64, odd head on 64:128); the scores matmuls use K=64
            # row-tiles at base partitions 0 and 64, which run concurrently
            # on disjoint PE array cells (tile_position row tiling).
            qT = big.tile([128, NPAIR, S], BF16, tag="qT")
            kT = big.tile([128, NPAIR, S], BF16, tag="kT")
            v_aug = big.tile([128, NKT, HPC * 65], BF16, tag="vaug")
            # own memory (not aliased to xqT): later pairs' q-projections are
            # interleaved into attention as fillers, so xqT stays live while
            # oT is being written
            oT = big.tile([128, NPAIR, S], BF16, tag="oT")

            # -------- work-chunk emitters --------------------------------
            def q_proj_sc(t, sc, pool=None):
                pool, tag = pool or (mmps, "mm")
                ps = pool.tile([128, SC], F32, tag=tag)
                for dk2 in range(NDT // 2):
                    nc.tensor.matmul(
                        ps[:],
                        wq_sb[:, 2 * dk2 : 2 * dk2 + 2, ds(t * 128, 128)],
                        xqT[:, 2 * dk2 : 2 * dk2 + 2, ds(sc * SC, SC)],
                        start=(dk2 == 0),
                        stop=(dk2 == NDT // 2 - 1),
                        perf_mode=DR,
                    )
                nc.vector.tensor_scalar_add(
                    qT[:, t, ds(sc * SC, SC)], ps[:], bq_col[:, t : t + 1]
                )

            def k_proj_sc(t, sc, pool=None):
                pool, tag = pool or (mmps, "mm")
                ps = pool.tile([128, SC], F32, tag=tag)
                for dk2 in range(NDT // 2):
                    nc.tensor.matmul(
                        ps[:],
                        wk_sb[:, 2 * dk2 : 2 * dk2 + 2, ds(t * 128, 128)],
                        xkT[:, 2 * dk2 : 2 * dk2 + 2, ds(sc * SC, SC)],
                        start=(dk2 == 0),
                        stop=(dk2 == NDT // 2 - 1),
                        perf_mode=DR,
                    )
                nc.vector.tensor_scalar_add(
                    kT[:, t, ds(sc * SC, SC)], ps[:], bk_col[:, t : t + 1]
                )

            vl_tiles = {}

            def v_dma_st(st):
                vl = vlpool.tile([128, NDT, 128], BF16, tag="vl")
                nc.sync.dma_start(
                    vl[:],
                    v_ext[:, ds(st * 128, 128)].rearrange("(t p) s -> p t s", p=128),
                )
                vl_tiles[st] = vl

            def v_proj_st(st):
                vl = vl_tiles.pop(st)
                ps = mmps.tile([128, DV], F32, tag="mm")
                for dk in range(NDT):
                    nc.tensor.matmul(
                        ps[:],
                        vl[:, dk, :],
                        wv_sb[:, dk, :],
                        start=(dk == 0),
                        stop=(dk == NDT - 1),
                    )
                dst = v_aug[:, st, :].rearrange("p (h c) -> p h c", c=65)
                nc.vector.tensor_copy(
                    dst[:, :, 0:64], ps[:].rearrange("p (h c) -> p h c", c=64)
                )
                nc.vector.memset(dst[:, :, 64:65], 1.0)

            def outproj_dt(sc, dt2, c0=0, c1=None):
                c1 = SC if c1 is None else c1
                w = c1 - c0
                po = mmps.tile([128, SC], F32, tag="mm")
                for ht in range(NPAIR):
                    nc.tensor.matmul(
                        po[:, 0:w],
                        wo_sb[:, ht, ds(dt2 * 128, 128)],
                        oT[:, ht, ds(sc * SC + c0, w)],
                        start=(ht == 0),
                        stop=(ht == NPAIR - 1),
                    )
                ost = stage.tile([128, SC], BF16, tag="ostage")
                nc.vector.tensor_copy(ost[:, 0:w], po[:, 0:w])
                nc.sync.dma_start(
                    out_ext[ds(dt2 * 128, 128), ds(sc * SC + c0, w)], ost[:, 0:w]
                )

            # -------- HAM warm-up ----------------------------------------
            # ~7us of dependency-free matmuls on the ones tile: keeps the PE
            # busy during the initial DMA-only window so the HAM activity
            # monitor un-throttles the clock before the real projections.
            for wi in range(18):
                wps = mmps.tile([128, SC], F32, tag="mm")
                for _ in range(2):
                    nc.tensor.matmul(
                        wps[:], ones_pad[:, 0:128], ones_pad[:],
                        start=True, stop=True,
                    )

            # -------- projections needed before attention(0) -------------
            # Rotate PSUM groups through the (still idle) attention pools so
            # the PE pipeline stays dense -- otherwise the ~1.4us slot-wait
            # gaps keep resetting the HAM activity window and the whole
            # projection phase runs at half clock.
            rot = [(mmps, "mm"), (scps, "sc"), (opool, "o")]
            ri = 0
            # minimal pre-work: attention(0, qc0) only needs q chunk 0 and
            # (progressively) kT; the rest stream in as fillers. The first
            # V-slice prefetches are issued here so they land before the
            # second input halves in the DMA queue.
            q_proj_sc(0, 0, pool=rot[0])
            k_proj_sc(0, 0, pool=rot[1])
            for st in range(3):
                v_dma_st(st)
            dma_xk_h2()

            # -------- attention ------------------------------------------
            # ACT (exp) is the long pole; every other PE-work chunk is
            # interleaved into the kt loop as "filler" so the in-order PE
            # queue never parks a long burst in front of the next scores.
            # WSCALE**2 undoes the host-side fp8 weight pre-scaling of Wq/Wk.
            SCALEF = SCALE / (WSCALE * WSCALE)

            deferred = []

            def normalize_one(o_sb, t, qc, hh, c0=0, c1=None):
                # Move the denominator row to partition 0 with a tiny
                # SBUF->SBUF DMA (partition-free, idle engines), broadcast,
                # then take the reciprocal on all 64 lanes in parallel --
                # the HW partition-broadcast only reads partition 0 (an AP
                # base of 64 yields NaN), and a [1, N] reciprocal runs
                # serially on one DVE lane.
                c1 = QC if c1 is None else c1
                w = c1 - c0
                dn = recpool.tile([64, QC], F32, tag="rec")
                if o_sb.space == bass.MemorySpace.PSUM:
                    # DMA cannot read PSUM; a [1, N] DVE copy does the row
                    # move (single lane, ~0.6us) without the sync-queue hop
                    nc.vector.tensor_copy(dn[0:1, 0:w], o_sb[64:65, c0:c1])
                else:
                    nc.sync.dma_start(dn[0:1, 0:w], o_sb[64:65, c0:c1])
                bc = recpool.tile([64, QC], F32, tag="rec")
                nc.gpsimd.partition_broadcast(bc[:, 0:w], dn[0:1, 0:w])
                # ~51 ULP, ~5x faster than reciprocal() (which costs ~3.3us
                # per 512 columns regardless of partition count)
                nc.vector.reciprocal_approx_fast(out=bc[:, 0:w], in_=bc[:, 0:w])
                nc.vector.tensor_mul(
                    oT[ds(hh * 64, 64), t, ds(qc * QC + c0, w)],
                    o_sb[0:64, c0:c1],
                    bc[:, 0:w],
                )

            def attention_pair(t):
                for qc in range(NQC):
                    # filler thunks interleaved after each scores/exp step;
                    # lag = how many k-tiles attn@V trails the exp stream
                    # (deep for the very first chunk so attn@V can wait for
                    # the V projection without stalling the exp feed)
                    fillers = {}
                    lag = 4
                    if t == 0 and qc == 0:
                        # V-slice DMA prefetch ~3 k-tiles ahead of its
                        # projection; projection one kt ahead of attn@V's lag
                        for st in range(3, NKT):
                            fillers.setdefault(min(st - 2, NKT - 1), []).append(
                                lambda st=st: v_dma_st(st)
                            )
                        for st in range(NKT):
                            fillers.setdefault(min(1 + st, NKT - 1), []).append(
                                lambda st=st: v_proj_st(st)
                            )
                        # rest of pair-0's K projection, each chunk one block
                        # ahead of the first scores k-tile that reads it
                        for j, slot in ((1, 2), (2, 5), (3, 9)):
                            fillers.setdefault(slot, []).append(
                                lambda j=j: k_proj_sc(0, j)
                            )
                        # xq second half: needed by q(0,2) emitted at kt 11
                        fillers.setdefault(8, []).append(dma_xq_h2)
                    if t == 0 and qc == 1:
                        fillers.setdefault(0, []).append(dma_wo)
                    if t == 0 and qc < NQC - 1:
                        # pair-0's next q chunk, late in the current chunk
                        fillers.setdefault(11, []).append(
                            lambda sc=qc + 1: q_proj_sc(0, sc)
                        )
                    if t + 1 < NPAIR and qc in (1, 2):
                        # next pair's Q projection, two chunks per qc chunk
                        for i in range(2):
                            fillers.setdefault(3 + 8 * i, []).append(
                                lambda tt=t + 1, sc=2 * (qc - 1) + i: q_proj_sc(tt, sc)
                            )
                    if t + 1 < NPAIR and qc == NQC - 1:
                        for i in range(NSC):
                            fillers.setdefault(2 + 4 * i, []).append(
                                lambda tt=t + 1, sc=i: k_proj_sc(tt, sc)
                            )
                    if t == NPAIR - 1 and qc > 0:
                        # kt >= 8 (block 4+) only: after BOTH deferred
                        # normalizations of the previous chunk (popped in
                        # block 1) have finished their DVE chains and written
                        # the oT slices these consume
                        for i in range(NDT):
                            fillers.setdefault(8 + i, []).append(
                                lambda sc=qc - 1, dt2=i: outproj_dt(sc, dt2)
                            )

                    oA = opool.tile([65, QC], F32, tag="o")
                    oB = opool.tile([65, QC], F32, tag="o")
                    pts = {}

                    def scores_exp(kt):
                        # two concurrent K=64 row-tiles: even head on array
                        # rows 0:64, odd head on rows 64:128 (tile_position
                        # inferred from the operands' base partitions)
                        sct = scps.tile([128, 2 * QC], F32, tag="sc")
                        nc.tensor.matmul(
                            sct[:, 0:QC],
                            kT[0:64, t, ds(kt * 128, 128)],
                            qT[0:64, t, ds(qc * QC, QC)],
                            start=True, stop=True,
                        )
                        nc.tensor.matmul(
                            sct[:, QC : 2 * QC],
                            kT[64:128, t, ds(kt * 128, 128)],
                            qT[64:128, t, ds(qc * QC, QC)],
                            start=True, stop=True,
                        )
                        pt = ptpool.tile([128, 2 * QC], BF16, tag="pt")
                        nc.scalar.activation(pt[:], sct[:], EXP, bias=0.0, scale=SCALEF)
                        pts[kt] = pt

                    def attn_v(kt):
                        pt = pts.pop(kt)
                        nc.tensor.matmul(
                            oA[:],
                            v_aug[:, kt, ds((2 * t) * 65, 65)],
                            pt[:, 0:QC],
                            start=(kt == 0),
                            stop=(kt == NKT - 1),
                        )
                        nc.tensor.matmul(
                            oB[:],
                            v_aug[:, kt, ds((2 * t + 1) * 65, 65)],
                            pt[:, QC : 2 * QC],
                            start=(kt == 0),
                            stop=(kt == NKT - 1),
                        )

                    # 2-kt blocks, mode-batched: per block the PE sees one
                    # 128-mode burst (attn@V pair + fillers) then one 64-mode
                    # burst (2x row-tiled scores pairs), so tile-mode drains
                    # happen twice per block instead of twice per kt. Scores
                    # go LAST in the block: they are the instructions that can
                    # stall on the ACT exp (scps WAR), and the in-order PE
                    # queue must have the fillers in front of them.
                    # In the very first chunk the fillers are DMA-gated
                    # (V slices still landing), so scores go FIRST there to
                    # keep the exp stream fed; everywhere else scores go last
                    # because they are the ops that stall on the ACT (scps
                    # WAR) and the in-order PE queue must keep fillers in
                    # front of them.
                    first = t == 0 and qc == 0
                    for b in range(NKT // 2):
                        kt0, kt1 = 2 * b, 2 * b + 1
                        if kt0 - lag >= 0:
                            attn_v(kt0 - lag)
                            attn_v(kt1 - lag)
                        if first:
                            scores_exp(kt0)
                            scores_exp(kt1)
                        for kt in (kt0, kt1):
                            for f in fillers.get(kt, ()):
                                f()
                        if b == 1:
                            while deferred:
                                normalize_one(*deferred.pop(0))
                        if not first:
                            scores_exp(kt0)
                            scores_exp(kt1)
                    for kt in range(max(0, NKT - lag), NKT):
                        attn_v(kt)

                    # copy both accumulators PSUM->SBUF now (frees the
                    # banks for the next chunk within ~1us); the recip/
                    # broadcast/mul chains are DEFERRED into the next
                    # chunk's loop so they soak up DVE idle time there
                    # instead of stalling this boundary
                    for o_ps, hh in ((oA, 0), (oB, 1)):
                        if t == NPAIR - 1 and qc == NQC - 1:
                            # final chunk: normalize straight from PSUM (no
                            # further matmuls will claim these banks), which
                            # drops two DVE copies from the closing chain
                            deferred.append((o_ps, t, qc, hh))
                        else:
                            o_sb = ounpool.tile([65, QC], F32, tag="oun")
                            nc.vector.tensor_copy(o_sb[:], o_ps[:])
                            deferred.append((o_sb, t, qc, hh))

            for t in range(NPAIR):
                attention_pair(t)
            # Tail: the final chunk's two head-norms are still deferred.
            # Process them in two column waves (384 then 128) with the
            # output projection interleaved, so the last serial
            # norm->outproj->DMA chain only covers 128 columns.
            SPLIT = 384
            final = list(deferred)
            deferred.clear()
            for args in final:
                normalize_one(*args, 0, SPLIT)
            # norm-b emitted before the first wave: its DVE/gpsimd chain
            # overlaps the 384-wave matmuls instead of queueing behind
            # their staging copies, so the 128-wave starts immediately
            for args in final:
                normalize_one(*args, SPLIT, QC)
            for dt2 in range(NDT):
                outproj_dt(NQC - 1, dt2, 0, SPLIT)
            for dt2 in range(NDT):
                outproj_dt(NQC - 1, dt2, SPLIT, SC)

    nc.finalize()
    return nc


_NC_CACHE = {}


def _get_nc():
    if "nc" not in _NC_CACHE:
        _NC_CACHE["nc"] = build_attn_core(S=S, D=D, HPC=HPC, HD=HD)
    return _NC_CACHE["nc"]


def _make_in_maps(query, key, value, Wq, bq, Wk, bk, Wv, bv, Wo):
    bf = ml_dtypes.bfloat16
    f8 = ml_dtypes.float8_e4m3fn
    ws = 16.0  # must match kernel WSCALE
    in_maps = []
    for c in range(N_CORES):
        b, hg = c // 2, c % 2
        sl = slice(hg * DV, (hg + 1) * DV)
        in_maps.append(dict(
            queryT=np.ascontiguousarray(query[b].T).astype(f8),
            keyT=np.ascontiguousarray(key[b].T).astype(f8),
            valueT=np.ascontiguousarray(value[b].T).astype(bf),
            Wq=(np.ascontiguousarray(Wq[:, sl]) * ws).astype(f8),
            Wk=(np.ascontiguousarray(Wk[:, sl]) * ws).astype(f8),
            Wv=np.ascontiguousarray(Wv[:, sl]).astype(bf),
            Wo=np.ascontiguousarray(Wo[sl, :]).astype(bf),
            bq=(np.ascontiguousarray(bq[sl]) * ws).astype(bf),
            bk=(np.ascontiguousarray(bk[sl]) * ws).astype(bf),
        ))
    return in_maps


def _assemble(results, bo):
    out = np.empty((B, S, D), dtype=np.float32)
    for b in range(B):
        part = (results[2 * b]["out"].astype(np.float32)
                + results[2 * b + 1]["out"].astype(np.float32))   # [D, S]
        out[b] = part.T + bo
    return out


def run(inputs, trace=False):
    """Run on 8 cores; returns (output, BassKernelResults)."""
    from concourse.bass_utils import run_bass_kernel_spmd

    inputs = {k: np.asarray(v) for k, v in inputs.items()}
    nc = _get_nc()
    in_maps = _make_in_maps(
        inputs["query"], inputs["key"], inputs["value"],
        inputs["Wq"], inputs["bq"], inputs["Wk"], inputs["bk"],
        inputs["Wv"], inputs["bv"], inputs["Wo"],
    )
    res = run_bass_kernel_spmd(
        nc, in_maps, core_ids=list(range(N_CORES)), trace=trace
    )
    # bv is folded through the output projection: (O/d + bv) @ Wo + bo
    # == (O/d) @ Wo + (bv @ Wo + bo), so the device never sees bv.
    bo_eff = (
        np.asarray(inputs["bv"], dtype=np.float64)
        @ np.asarray(inputs["Wo"], dtype=np.float64)
        + np.asarray(inputs["bo"], dtype=np.float64)
    ).astype(np.float32)
    out = _assemble(res.results, bo_eff)
    return out, res


def kernel(**inputs) -> np.ndarray:
    out, _ = run(inputs, trace=False)
    return out

